# revision 13
# baseline (speedup 1.0000x reference)
"""TRN2 Bass kernel for nn_Base_1348619731207 (gnn_message_passing).

Model:
  graph_out = MLP_graph(mean_pool(x, batch))            # [B, G]
  node_out[b, n] = MLP_node_n(x[b, n, :])               # per-node MLPs, [B, N]
  out = concat([graph_out, node_out], axis=1)           # [B, G + N]

Sharding (8 cores): expert-parallel over the node dim N (64 nodes/core,
per-node head weights sliced with their nodes) + graph-parallel pooling
(16 graphs/core stream their own x rows for the mean-pool + graph head).
No collectives.

Memory regime: node stream (x slice + w1 + w2) is bf16 (halves HBM
traffic and PE moving cycles vs fp32r; bf16 runs 1 cycle/row even at
128-wide moving). The pooling stream (x rows + segment-mean indicator)
is fp8 e3m4; the indicator is pre-scaled by 512 so count-reciprocals sit
in e3m4's normal range, undone via the relu's scale (relu is positively
homogeneous). Graph-head weights are bf16.

Schedule: per-node loop is software-pipelined (L1(s) | L2(s-2) | L3(s-4))
so the PE never waits on relu; relus are spread over ACT/DVE/GpSimd.
DMAs ride two HWDGE queues (sync + scalar) so descriptor generation on
one queue overlaps transfers of the other; constants are packed into
three DMAs so the first stream tile lands within ~3 us.
"""

import numpy as np
import ml_dtypes

import concourse.bass as bass
import concourse.mybir as mybir
from concourse import bacc
from concourse.bass_utils import run_bass_kernel_spmd
from concourse.masks import make_identity
from concourse.tile import TileContext

F32 = mybir.dt.float32
BF16 = mybir.dt.bfloat16
FP8 = mybir.dt.float8e3            # e3m4
RELU = mybir.ActivationFunctionType.Relu
IDENT = mybir.ActivationFunctionType.Identity
ADD = mybir.AluOpType.add
MAX = mybir.AluOpType.max

NP_BF16 = np.dtype(ml_dtypes.bfloat16)
NP_FP8 = np.dtype(ml_dtypes.float8_e3m4)

B, N, H = 128, 512, 256          # graphs, nodes/graph, hidden
DS, D1, D2, G = 128, 256, 128, 32
NCORES = 8
NPC = N // NCORES                # 64 nodes per core
GPC = B // NCORES                # 16 graphs per core
PT = 68                          # pooling row tiles per core (68*128 = 8704 rows)
XGPACK = 4                       # pooling row tiles packed per DMA
NXG = PT // XGPACK               # 17 pooling DMA tiles
IND_SCALE = 512.0                # keeps 1/count in e3m4 normal range

# f32 const pack columns
CB1, CB2, CB3 = 0, 128, 192
CGB1, CGB2, CGB3, CGB4, CGB5 = 256, 257, 258, 260, 261
CF32 = 262
# bf16 const pack columns
CW3, CGW1, CGW2, CGW3, CGW4, CGW5 = 0, 64, 320, 448, 704, 960
CBF = 992

_CACHE = {}


def _build_nc():
    nc = bacc.Bacc("TRN2", target_bir_lowering=False, debug=False)

    # stream: per node 1024 cols = [xt(256: kh*128+b) | w1(512: (kh,mh)*128+m)
    # | w2(256: dh*128+m)], two nodes per DMA tile.
    st_d = nc.dram_tensor("st", [NPC // 2, 128, 2048], BF16, kind="ExternalInput")
    cf_d = nc.dram_tensor("cf", [128, CF32], F32, kind="ExternalInput")
    cb_d = nc.dram_tensor("cb", [128, CBF], BF16, kind="ExternalInput")
    xg_d = nc.dram_tensor("xg", [NXG, 128, 256 * XGPACK], FP8, kind="ExternalInput")
    ind_d = nc.dram_tensor("ind", [128, PT * GPC], FP8, kind="ExternalInput")

    nout_d = nc.dram_tensor("nout", [128, NPC], F32, kind="ExternalOutput")
    gout_d = nc.dram_tensor("gout", [G, GPC], F32, kind="ExternalOutput")

    with TileContext(nc) as tc:
        with (
            tc.tile_pool(name="const", bufs=1) as cst,
            tc.tile_pool(name="stream", bufs=8) as stp,
            tc.tile_pool(name="act", bufs=4) as actp,
            tc.tile_pool(name="h2p", bufs=6) as h2p,
            tc.tile_pool(name="xgp", bufs=6) as xgp,
            tc.tile_pool(name="psA", bufs=3, space=bass.MemorySpace.PSUM) as psA,
            tc.tile_pool(name="psB", bufs=3, space=bass.MemorySpace.PSUM) as psB,
            tc.tile_pool(name="psC", bufs=1, space=bass.MemorySpace.PSUM) as psC,
            tc.tile_pool(name="psD", bufs=1, space=bass.MemorySpace.PSUM) as psD,
        ):
            # --- packed constants + first tiles, split across both queues ---
            cft = cst.tile([128, CF32], F32)
            cbt = cst.tile([128, CBF], BF16)
            indt = cst.tile([128, PT * GPC], FP8)

            st_tiles = [None] * (NPC // 2)
            xg_tiles = [None] * NXG

            def dma_stream(i):
                st = stp.tile([128, 2048], BF16, tag="st")
                (nc.sync if i % 2 == 0 else nc.scalar).dma_start(st[:], st_d[i])
                st_tiles[i] = st

            def dma_xg(tt):
                xg = xgp.tile([128, 256 * XGPACK], FP8, tag="xg")
                nc.gpsimd.dma_start(xg[:], xg_d[tt])
                xg_tiles[tt] = xg

            dma_stream(0)
            dma_stream(1)
            nc.sync.dma_start(cft[:], cf_d[:])
            nc.gpsimd.dma_start(indt[:], ind_d[:])
            nc.scalar.dma_start(cbt[:], cb_d[:])
            dma_xg(0)
            dma_stream(2)
            dma_stream(3)
            dma_xg(1)

            zeros = cst.tile([128, 128], F32)
            nc.gpsimd.memset(zeros[:], 0.0)
            ident = cst.tile([128, 128], F32)
            make_identity(nc, ident[:])

            # node_out staging: L3 accumulates into one psum tile [b, n_loc]
            p3 = psC.tile([128, NPC], F32)
            nout_sb = cst.tile([128, NPC], F32)
            # pooling accumulator
            pp = psD.tile([GPC, 256], F32)

            def pool_tile(t):
                tt, q = divmod(t, XGPACK)
                nc.tensor.matmul(
                    pp[:],
                    indt[:, t * GPC:(t + 1) * GPC],
                    xg_tiles[tt][:, q * 256:(q + 1) * 256],
                    start=(t == 0), stop=(t == PT - 1),
                    skip_group_check=True,
                )

            h1s = [None] * NPC
            h2s = [None] * NPC

            def stage_l1(n):
                st = st_tiles[n // 2]
                base = (n % 2) * 1024
                p1 = psA.tile([128, 256], F32, tag="p1")
                for mh in range(2):
                    for kh in range(2):
                        nc.tensor.matmul(
                            p1[:, mh * 128:(mh + 1) * 128],
                            st[:, base + 256 + (kh * 2 + mh) * 128:
                                 base + 256 + (kh * 2 + mh + 1) * 128],
                            st[:, base + kh * 128:base + (kh + 1) * 128],
                            start=(kh == 0), stop=(kh == 1),
                        )
                h1 = actp.tile([128, 256], BF16, tag="h1")
                # split the two relu+bias chunks across ACT and DVE
                nc.scalar.activation(
                    h1[:, 0:128], p1[:, 0:128], RELU,
                    bias=cft[:, CB1 + 2 * n:CB1 + 2 * n + 1],
                )
                nc.vector.scalar_tensor_tensor(
                    h1[:, 128:256], p1[:, 128:256],
                    cft[:, CB1 + 2 * n + 1:CB1 + 2 * n + 2],
                    zeros[:],
                    ADD, MAX,
                )
                h1s[n] = h1

            def stage_l2(n):
                st = st_tiles[n // 2]
                base = (n % 2) * 1024
                h1 = h1s[n]
                p2 = psB.tile([128, 128], F32, tag="p2")
                for dh in range(2):
                    nc.tensor.matmul(
                        p2[:],
                        st[:, base + 768 + dh * 128:base + 768 + (dh + 1) * 128],
                        h1[:, dh * 128:(dh + 1) * 128],
                        start=(dh == 0), stop=(dh == 1),
                    )
                h2 = h2p.tile([128, 128], BF16, tag="h2")
                # L2 relu on DVE (ACT is loaded with L1 + DMA issue; GpSimd
                # cannot read PSUM)
                nc.vector.scalar_tensor_tensor(
                    h2[:], p2[:], cft[:, CB2 + n:CB2 + n + 1],
                    zeros[:], ADD, MAX,
                )
                h2s[n] = h2

            def stage_l3(n):
                nc.tensor.matmul(
                    p3[:, n:n + 1],
                    h2s[n],
                    cbt[:, CW3 + n:CW3 + n + 1],
                    start=True, stop=True,
                )
                h2s[n] = None

            gh_state = {}

            def gh_stage0():
                # relu(scale * pp) then transpose [GPC, 256] -> [256, GPC]
                xgr = actp.tile([GPC, 256], F32, tag="xgr")
                nc.scalar.activation(xgr[:], pp[:], RELU, scale=1.0 / IND_SCALE)
                xgt = actp.tile([128, 2 * GPC], BF16, tag="xgt")
                gh_state["xgr"], gh_state["xgt"] = xgr, xgt

            def gh_stage1(kh):
                xgr, xgt = gh_state["xgr"], gh_state["xgt"]
                ptr = psB.tile([128, GPC], F32, tag="p2")
                nc.tensor.transpose(
                    ptr[:], xgr[:, kh * 128:(kh + 1) * 128], ident[:GPC, :GPC]
                )
                nc.vector.tensor_copy(xgt[:, kh * GPC:(kh + 1) * GPC], ptr[:])

            def gh_stage2():
                xgt = gh_state["xgt"]
                # layer 1: relu(x_graph) @ gs_w1 + gs_b1   (no relu after)
                g1 = psB.tile([128, GPC], F32, tag="p2")
                for kh in range(2):
                    nc.tensor.matmul(
                        g1[:], cbt[:, CGW1 + kh * 128:CGW1 + (kh + 1) * 128],
                        xgt[:, kh * GPC:(kh + 1) * GPC],
                        start=(kh == 0), stop=(kh == 1),
                    )
                a1 = actp.tile([128, GPC], BF16, tag="ga")
                nc.scalar.activation(a1[:], g1[:], IDENT,
                                     bias=cft[:, CGB1:CGB1 + 1])
                gh_state["a1"] = a1

            def gh_stage3():
                # layer 2: relu(a1 @ gs_w2 + gs_b2)
                g2 = psB.tile([128, GPC], F32, tag="p2")
                nc.tensor.matmul(g2[:], cbt[:, CGW2:CGW2 + 128],
                                 gh_state["a1"][:], start=True, stop=True)
                a2 = actp.tile([128, GPC], BF16, tag="ga")
                nc.scalar.activation(a2[:], g2[:], RELU,
                                     bias=cft[:, CGB2:CGB2 + 1])
                a3 = actp.tile([128, 2 * GPC], BF16, tag="ga3")
                gh_state["a2"], gh_state["a3"] = a2, a3

            def gh_stage4(mh):
                # layer 3: relu(a2 @ gh_w1 + gh_b1)  (D1 = 256 -> two M halves)
                a3 = gh_state["a3"]
                g3 = psB.tile([128, GPC], F32, tag="p2")
                nc.tensor.matmul(
                    g3[:], cbt[:, CGW3 + mh * 128:CGW3 + (mh + 1) * 128],
                    gh_state["a2"][:],
                    start=True, stop=True,
                )
                nc.scalar.activation(
                    a3[:, mh * GPC:(mh + 1) * GPC], g3[:], RELU,
                    bias=cft[:, CGB3 + mh:CGB3 + mh + 1],
                )

            def gh_stage5():
                # layer 4: relu(a3 @ gh_w2 + gh_b2)
                a3 = gh_state["a3"]
                g4 = psB.tile([128, GPC], F32, tag="p2")
                for kh in range(2):
                    nc.tensor.matmul(
                        g4[:], cbt[:, CGW4 + kh * 128:CGW4 + (kh + 1) * 128],
                        a3[:, kh * GPC:(kh + 1) * GPC],
                        start=(kh == 0), stop=(kh == 1),
                    )
                a4 = actp.tile([128, GPC], BF16, tag="ga")
                nc.scalar.activation(a4[:], g4[:], RELU,
                                     bias=cft[:, CGB4:CGB4 + 1])
                gh_state["a4"] = a4

            def gh_stage6():
                # layer 5: a4 @ gh_w3 + gh_b3
                g5 = psB.tile([G, GPC], F32, tag="p2")
                nc.tensor.matmul(g5[:], cbt[:, CGW5:CGW5 + G],
                                 gh_state["a4"][:], start=True, stop=True)
                gout_sb = actp.tile([G, GPC], F32, tag="gout")
                nc.scalar.activation(gout_sb[:], g5[:], IDENT,
                                     bias=cft[:G, CGB5:CGB5 + 1])
                nc.scalar.dma_start(gout_d[:], gout_sb[:])

            gh_sched = {
                36: gh_stage0,
                38: lambda: gh_stage1(0),
                40: lambda: gh_stage1(1),
                42: gh_stage2,
                44: gh_stage3,
                46: lambda: gh_stage4(0),
                48: lambda: gh_stage4(1),
                50: gh_stage5,
                52: gh_stage6,
            }

            # --- software-pipelined node loop: L1(s) | L2(s-2) | L3(s-4).
            # Pooling runs 2 tiles/step starting s=2 (pp closes s=35); the
            # graph head is interleaved at s=37 so its serial tail overlaps
            # the remaining node stream. Stream pairs and xg tiles are
            # prefetched 4 pairs / 2 tiles ahead on alternating queues. ---
            pool_t = 0
            for s in range(NPC + 4):
                p = 4 + s // 2
                if s % 2 == 0 and p < NPC // 2:
                    dma_stream(p)
                tt = 2 + s // 2
                if s % 2 == 1 and tt < NXG:
                    dma_xg(tt)
                if s in gh_sched:
                    gh_sched[s]()
                if s < NPC:
                    stage_l1(s)
                if 2 <= s < NPC + 2:
                    stage_l2(s - 2)
                if s >= 4:
                    stage_l3(s - 4)
                while pool_t < min(PT, 2 * max(0, s - 1)):
                    pool_tile(pool_t)
                    pool_t += 1

            nc.vector.tensor_add(nout_sb[:], p3[:], cft[:, CB3:CB3 + 64])
            nc.sync.dma_start(nout_d[:], nout_sb[:])

    nc.compile()
    return nc


def _prep_core_inputs(c, x, batch, lo_hi, inv_counts,
                      nh_w1, nh_w2, nh_w3, cf_base, cb_base):
    ns = slice(c * NPC, (c + 1) * NPC)
    xv = x.reshape(B, N, H)

    # packed bf16 node stream: [n, p, 1024] -> pairs [n/2, p, 2048]
    S = np.empty((NPC, 128, 1024), np.float32)
    S[:, :, 0:256] = (
        xv[:, ns, :]                              # [b, n, h]
        .reshape(B, NPC, 2, 128)                  # b, n, kh, p
        .transpose(1, 3, 2, 0)                    # n, p, kh, b
        .reshape(NPC, 128, 256)
    )
    S[:, :, 256:768] = (
        nh_w1[ns]                                 # [n, h, d1]
        .reshape(NPC, 2, 128, 2, 128)             # n, kh, p, mh, m
        .transpose(0, 2, 1, 3, 4)                 # n, p, kh, mh, m
        .reshape(NPC, 128, 512)
    )
    S[:, :, 768:1024] = (
        nh_w2[ns]                                 # [n, d1, d2]
        .reshape(NPC, 2, 128, 128)                # n, dh, p, m
        .transpose(0, 2, 1, 3)                    # n, p, dh, m
        .reshape(NPC, 128, 256)
    )
    st = np.ascontiguousarray(
        S.astype(NP_BF16)
        .reshape(NPC // 2, 2, 128, 1024)
        .transpose(0, 2, 1, 3)
        .reshape(NPC // 2, 128, 2048)
    )

    cb = cb_base.copy()
    cb[:, CW3:CW3 + NPC] = nh_w3[ns, :, 0].T.astype(NP_BF16)

    # pooling rows for graphs [GPC*c, GPC*(c+1)), fp8 e3m4
    lo, hi = lo_hi[c]
    nrows = hi - lo
    xg = np.zeros((PT * 128, 256), NP_FP8)
    xg[:nrows] = x[lo:hi].astype(NP_FP8)
    xg = np.ascontiguousarray(
        xg.reshape(NXG, XGPACK, 128, 256)
        .transpose(0, 2, 1, 3)
        .reshape(NXG, 128, 256 * XGPACK)
    )
    ind = np.zeros((PT * 128, GPC), np.float32)
    gl = batch[lo:hi] - GPC * c
    ind[np.arange(nrows), gl] = IND_SCALE * inv_counts[batch[lo:hi]]
    ind = np.ascontiguousarray(
        ind.reshape(PT, 128, GPC).transpose(1, 0, 2).reshape(128, PT * GPC)
    ).astype(NP_FP8)

    return {"st": st, "cf": cf_base, "cb": cb, "xg": xg, "ind": ind}


def kernel(x, batch, gs_w1, gs_b1, gs_w2, gs_b2,
           gh_w1, gh_b1, gh_w2, gh_b2, gh_w3, gh_b3,
           nh_w1, nh_b1, nh_w2, nh_b2, nh_w3, nh_b3):
    x = np.asarray(x, np.float32)
    batch = np.asarray(batch, np.int32)

    counts = np.bincount(batch, minlength=B).astype(np.float32)
    inv_counts = np.where(counts > 0, 1.0 / np.maximum(counts, 1), 0.0).astype(
        np.float32
    )
    # row ranges per core (batch is sorted); must fit in the padded tile count
    bounds = np.searchsorted(batch, np.arange(0, B + 1, GPC))
    lo_hi = [(int(bounds[c]), int(bounds[c + 1])) for c in range(NCORES)]
    assert all(hi - lo <= PT * 128 for lo, hi in lo_hi), "graph slice too large"

    nh_w1 = np.asarray(nh_w1, np.float32)
    nh_w2 = np.asarray(nh_w2, np.float32)
    nh_w3 = np.asarray(nh_w3, np.float32)
    nh_b1 = np.asarray(nh_b1, np.float32)
    nh_b2 = np.asarray(nh_b2, np.float32)
    nh_b3 = np.asarray(nh_b3, np.float32)

    # bf16 const pack (graph head weights; w3 slot filled per-core)
    cb_base = np.zeros((128, CBF), NP_BF16)
    cb_base[:, CGW1:CGW1 + 256] = (
        np.asarray(gs_w1, np.float32).reshape(2, 128, 128)
        .transpose(1, 0, 2).reshape(128, 256).astype(NP_BF16)
    )
    cb_base[:, CGW2:CGW2 + 128] = np.asarray(gs_w2, np.float32).astype(NP_BF16)
    cb_base[:, CGW3:CGW3 + 256] = np.asarray(gh_w1, np.float32).astype(NP_BF16)
    cb_base[:, CGW4:CGW4 + 256] = (
        np.asarray(gh_w2, np.float32).reshape(2, 128, 128)
        .transpose(1, 0, 2).reshape(128, 256).astype(NP_BF16)
    )
    cb_base[:, CGW5:CGW5 + G] = np.asarray(gh_w3, np.float32).astype(NP_BF16)

    # f32 const pack (per-core b1/b2/b3 slices differ)
    def cf_for_core(c):
        ns = slice(c * NPC, (c + 1) * NPC)
        cf = np.zeros((128, CF32), np.float32)
        cf[:, CB1:CB1 + 128] = (
            nh_b1[ns].reshape(NPC, 2, 128).transpose(2, 0, 1).reshape(128, 2 * NPC)
        )
        cf[:, CB2:CB2 + 64] = nh_b2[ns].T
        cf[:, CB3:CB3 + 64] = np.broadcast_to(nh_b3[ns].reshape(1, NPC), (128, NPC))
        cf[:, CGB1] = np.asarray(gs_b1, np.float32)
        cf[:, CGB2] = np.asarray(gs_b2, np.float32)
        cf[:, CGB3:CGB3 + 2] = np.asarray(gh_b1, np.float32).reshape(2, 128).T
        cf[:, CGB4] = np.asarray(gh_b2, np.float32)
        cf[:G, CGB5] = np.asarray(gh_b3, np.float32)
        return cf

    if "nc" not in _CACHE:
        _CACHE["nc"] = _build_nc()
    nc = _CACHE["nc"]

    in_maps = [
        _prep_core_inputs(c, x, batch, lo_hi, inv_counts,
                          nh_w1, nh_w2, nh_w3, cf_for_core(c), cb_base)
        for c in range(NCORES)
    ]

    res = run_bass_kernel_spmd(nc, in_maps, core_ids=list(range(NCORES)))
    _CACHE["last_result"] = res

    out = np.empty((B, G + N), np.float32)
    for c in range(NCORES):
        out[GPC * c:GPC * (c + 1), :G] = res.results[c]["gout"].T
        out[:, G + NPC * c:G + NPC * (c + 1)] = res.results[c]["nout"]
    return out
